# revision 22
# baseline (speedup 1.0000x reference)
"""Trainium2 Bass kernel for CategorySpecificLinear (MoE-style routed linear).

out[i] = x[i] @ W[cat_ids[i]] + b[cat_ids[i]]
  x: [64, 256, 1024] f32, cat_ids: [64] int, W: [16, 1024, 4096] f32,
  b: [16, 4096] f32  ->  out: [64, 256, 4096] f32

Strategy (expert-parallel routing, one shared program for all cores):
  * Host groups batch rows by cat_id and, when possible, gives every core
    the SAME (r1, r2, ...) slot signature (e.g. (4,3,1)): all 8 cores then
    run one byte-identical Bass program (single compile, one quality-gated
    schedule draw). Falls back to a per-core annealed plan otherwise.
  * x is transposed host-side so the contraction dim is the SBUF partition
    dim; all of x stays SBUF-resident; W is host-prepped per
    (expert, quarter) into contiguous [128, 8192] blocks so each W DMA is
    2D with 16KB rows.
  * Matmuls run in fp16 (fp32 PSUM accumulation; rel err ~5e-4 incl the
    fp16 output). fp8 DoubleRow (2x PE rate) was measured at 3.8e-2
    error on these inputs -- over the 2e-2 gate -- and any hi/lo
    compensation costs back the speedup, so fp16 is the fastest legal
    dtype; the kernel is PE-bound at ~216ns per N=512 matmul.
  * Startup (the only non-dense phase): ~12 warm-up matmuls on memset-ones
    data un-throttle the PE clock (HAM 1.2->2.4GHz) while the first
    input slices stream in; the first quarter is emitted kk-major over
    batches of 8 psum tiles so matmuls start after ~500KB of input; the
    supply is ring-serialized (x halves + first-quarter W k-slices on two
    2KB-descriptor queues, large W blocks strictly behind x on sync) since
    concurrent 16KB-descriptor bursts starve a 2KB queue to ~30GB/s.
  * The scheduled BIR is validated (exact DMA bytes, warm-ups first,
    bounded quarter reordering) and rebuilt with perturbed scheduler
    pacing on a bad draw -- the Tile list scheduler is nondeterministic
    across processes and a bad draw costs ~5-10us.
  * Output is written fp16 (halves write traffic; negligible error).
"""

import hashlib
import os
import pickle

import numpy as np

import concourse.bass as bass
import concourse.mybir as mybir

F32 = mybir.dt.float32
F16 = mybir.dt.float16
FP16 = mybir.dt.float16

NCORES = 8
SEQ = 256
KDIM = 1024
NDIM = 4096
KT = KDIM // 128   # 8 k-tiles
NQ = 4             # hidden-dim quarters of 1024
NPQ = 2            # 512-wide psum slices per quarter
MAX_ROWS = 10

_NEFF_CACHE_DIR = "/tmp/bass_neff_cache"


# ---------------------------------------------------------------- BIR fixup

def _fix_multi_waits(nc, max_waits=1):
    """The walrus build here rejects instructions carrying more than one
    sync-wait command; split extra waits onto single-wait NOPs inserted
    before the instruction on the same engine (same-engine waits execute
    in order, so this is semantics-preserving)."""
    for f in nc.m.functions:
        for blk in f.blocks:
            il = blk.instructions
            i = 0
            while i < len(il):
                inst = il[i]
                si = getattr(inst, "sync_info", None)
                if si is not None and len(si.on_wait) > max_waits:
                    waits = list(si.on_wait)
                    keep, extra = waits[-max_waits:], waits[:-max_waits]
                    for w in extra:
                        nop = mybir.InstNoOp(
                            name=nc.get_next_instruction_name(),
                            sync_info=mybir.SyncInfo(on_wait=[w], on_update=[]),
                            bass_nofuse=True,
                            engine=inst.engine,
                        )
                        nc.register_instruction(nop, overwrite=True)
                        il.insert(i, nop)
                        i += 1
                    inst.sync_info = mybir.SyncInfo(
                        on_wait=keep, on_update=list(si.on_update)
                    )
                i += 1


# ------------------------------------------------------------ program build

def _build_program(group_rows, dtype=FP16, salt=0):
    """group_rows: tuple of rows-per-expert-slot. The core computes, for
    each slot s, x_rows(s) @ W_slot(s) over all 4 hidden quarters.

    W arrives host-prepped as [u*4*128, 8192]: row-block (s*4+nq)*128 holds
    that expert-quarter as [p, kk*1024+f] (k = kk*128+p), so every W DMA is
    a plain 2D copy with 16KB-contiguous rows (128 descriptors).

    Supply rides three independent DMA rings so streams never serialize:
    x k-slices on sync, first-quarter W k-slices on scalar (ahead of the
    output DMAs), whole-quarter W blocks on gpsimd.

    `salt` perturbs the scheduler-pacing constants; _get_exec retries with
    different salts until the scheduled BIR passes _validate_nc (the Tile
    list scheduler is not deterministic across processes and occasionally
    draws a schedule with duplicated DMAs or deep compute reordering)."""
    from concourse import tile

    r_total = sum(group_rows)
    M = SEQ * r_total
    M2 = M // 2
    u = len(group_rows)
    nc = bass.Bass(enable_partition_id=False)
    xt_d = nc.declare_dram_parameter("xt", [KDIM, M], dtype, isOutput=False)
    w_d = nc.declare_dram_parameter(
        "w", [u * NQ * 128, KT * 1024], dtype, isOutput=False
    )
    out_d = nc.declare_dram_parameter("out", [NQ * M, 1024], F16, isOutput=True)

    # ~one N=512 matmul in ms, to pace scheduler phases
    MM_MS = 0.000220 * (1.0 + 0.017 * salt)

    with tile.TileContext(nc) as tc:
        with (
            tc.tile_pool(name="xt", bufs=1) as xt_pool,
            tc.tile_pool(name="wq", bufs=4) as w_pool,
            tc.tile_pool(name="ostage", bufs=6) as o_pool,
            tc.tile_pool(name="psum", bufs=8, space="PSUM") as p_pool,
        ):
            # Startup supply: x first-half k-slices stream on sync while the
            # first-quarter W k-slices stream on scalar — two parallel
            # 2KB-descriptor queues, so each (x, W) slice pair arrives at
            # ~the matmul consumption rate. Every LARGE (16KB-descriptor)
            # W-block DMA stays on the sync ring BEHIND the x slices:
            # big-descriptor bursts win HBM arbitration and would starve a
            # concurrent 2KB stream to ~30GB/s (measured), so they must
            # never run on a separate queue while x/wqf are still loading.
            # Scalar carries wqf then the output DMAs.
            # Warm-tile memset first: it shares the gpsimd engine with the
            # wqf DMA triggers below and must not queue behind them.
            warm = xt_pool.tile([128, 640], dtype, tag="warm", name="warmsrc")
            nc.gpsimd.memset(warm[:], 1.0)

            xts = [[None] * KT, [None] * KT]
            wqf = [None] * KT
            for kk in range(KT):
                t0 = xt_pool.tile([128, M2], dtype, tag=f"xts0_{kk}",
                                  name=f"xts0_{kk}")
                nc.sync.dma_start(out=t0[:], in_=xt_d[kk * 128:(kk + 1) * 128, 0:M2])
                xts[0][kk] = t0
                wt = w_pool.tile([128, 1024], dtype, tag=f"wqf{kk}",
                                 name=f"wqf{kk}")
                nc.gpsimd.dma_start(
                    out=wt[:], in_=w_d[0:128, kk * 1024:(kk + 1) * 1024]
                )
                wqf[kk] = wt

            # PE pre-warm: matmuls on memset-ones data run while the first
            # input slices stream in, so the HAM un-throttles (1.2->2.4GHz)
            # before the first real matmul instead of during the first ~16.
            # (All-zero operands leave the datapath idle and do not register
            # as activity, so the fill value must be non-zero.)
            ps_warm = p_pool.tile([128, 512], F32, tag="psum", name="ps_warm")
            for i in range(12):
                nc.tensor.matmul(
                    ps_warm[:], warm[:, 0:128], warm[:, 128:640],
                    start=(i == 0), stop=(i == 11),
                )

            def wq_dma(s, nq):
                t = w_pool.tile(
                    [128, KT * 1024], dtype, tag="wq", name=f"wq{s}_{nq}"
                )
                row = (s * NQ + nq) * 128
                nc.sync.dma_start(out=t[:], in_=w_d[row:row + 128, :])
                return t

            wq01 = wq_dma(0, 1)

            for kk in range(KT):
                t1 = xt_pool.tile([128, M2], dtype, tag=f"xts1_{kk}",
                                  name=f"xts1_{kk}")
                nc.sync.dma_start(out=t1[:], in_=xt_d[kk * 128:(kk + 1) * 128, M2:M])
                xts[1][kk] = t1

            def x_slice(kk, moff):
                h, off = divmod(moff, M2)
                return xts[h][kk][:, off:off + 128]

            cum_ms = [0.002]

            def pin(n_mms):
                tc.tile_set_cur_wait(cum_ms[0])
                cum_ms[0] += n_mms * MM_MS

            # ---- first quarter of expert 0: kk-major over batches of 8
            # psum tiles, so each arriving (x k-slice, W k-slice) pair
            # unlocks 8 matmuls and the in-order PE never waits for the
            # full x load.
            r0 = group_rows[0]
            tiles_q0 = [(mt, n2) for mt in range(2 * r0) for n2 in range(NPQ)]
            ost_q0 = {}
            for bstart in range(0, len(tiles_q0), 8):
                batch = tiles_q0[bstart:bstart + 8]
                pin(len(batch) * KT)
                ps = {}
                for kk in range(KT):
                    for (mt, n2) in batch:
                        if kk == 0:
                            ps[(mt, n2)] = p_pool.tile(
                                [128, 512], F32, tag="psum",
                                name=f"ps0_0_{mt}_{n2}",
                            )
                        nc.tensor.matmul(
                            ps[(mt, n2)][:],
                            x_slice(kk, mt * 128),
                            wqf[kk][:, n2 * 512:(n2 + 1) * 512],
                            start=(kk == 0),
                            stop=(kk == KT - 1),
                        )
                        if kk == KT - 1:
                            if mt not in ost_q0:
                                ost_q0[mt] = o_pool.tile(
                                    [128, 1024], F16, tag="ostage",
                                    name=f"os0_0_{mt}",
                                )
                            nc.vector.tensor_copy(
                                ost_q0[mt][:, n2 * 512:(n2 + 1) * 512],
                                ps[(mt, n2)][:],
                            )
                            if n2 == NPQ - 1:
                                nc.scalar.dma_start(
                                    out=out_d[mt * 128:mt * 128 + 128, :],
                                    in_=ost_q0[mt][:],
                                )

            # ---- all remaining (expert, quarter) pairs: standard kk-minor
            # per-psum-tile emission with whole-quarter W tiles prefetched
            # on the sync ring (DMA triggers pinned ~2 quarters early so
            # the scheduler keeps the prefetch ahead of compute).
            m_base = 0
            for s, rs in enumerate(group_rows):
                for nq in range(NQ):
                    if s == 0 and nq == 0:
                        continue
                    if s == 0 and nq == 1:
                        wq = wq01
                    else:
                        with tc.tile_wait_until(max(0.002, cum_ms[0] - 0.030)):
                            wq = wq_dma(s, nq)
                    pin(2 * rs * NPQ * KT)
                    for mt in range(2 * rs):
                        moff = m_base + mt * 128
                        ost = o_pool.tile(
                            [128, 1024], F16, tag="ostage", name=f"os{s}_{nq}_{mt}"
                        )
                        for n2 in range(NPQ):
                            pst = p_pool.tile(
                                [128, 512], F32, tag="psum",
                                name=f"ps{s}_{nq}_{mt}_{n2}",
                            )
                            for kk in range(KT):
                                nc.tensor.matmul(
                                    pst[:],
                                    x_slice(kk, moff),
                                    wq[:, kk * 1024 + n2 * 512:
                                       kk * 1024 + (n2 + 1) * 512],
                                    start=(kk == 0),
                                    stop=(kk == KT - 1),
                                )
                            nc.vector.tensor_copy(
                                ost[:, n2 * 512:(n2 + 1) * 512], pst[:]
                            )
                        nc.scalar.dma_start(
                            out=out_d[nq * M + moff:nq * M + moff + 128, :],
                            in_=ost[:],
                        )
                m_base += SEQ * rs
    _fix_multi_waits(nc)
    return nc


# ------------------------------------------------------------- build gating

def _validate_nc(nc, group_rows):
    """Sanity-check the *scheduled* BIR: exact DMA bytes per tensor class
    (catches scheduler-duplicated loads) and a bounded compute-reorder
    window (no quarter may start before the quarter three back has fully
    finished). Returns a list of problems (empty = good schedule)."""
    r_total = sum(group_rows)
    M = SEQ * r_total
    u = len(group_rows)
    probs = []

    def ap_bytes(ap):
        n = 1
        for pair in ap.ap:
            n *= pair[1]
        return n * mybir.dt.size(ap.dtype)

    dma_bytes = {"xts": 0, "wqf": 0, "wq": 0, "out": 0}
    dma_count = 0
    mm_quarters = []
    quarter_order = {"wqf": 0}
    qi = 1
    for s in range(u):
        for nq in range(NQ):
            if s == 0 and nq == 0:
                continue
            quarter_order[f"wq{s}_{nq}"] = qi
            qi += 1

    import re as _re
    for blk in nc.m.functions[0].blocks:
        for inst in blk.instructions:
            cls = inst.__class__.__name__
            if cls == "InstDMACopy":
                dma_count += 1
                ref = inst.outs[0].memref
                base = _re.sub(r"_\d+$", "", ref)
                if base.startswith("xts"):
                    dma_bytes["xts"] += ap_bytes(inst.outs[0])
                elif base.startswith("wqf"):
                    dma_bytes["wqf"] += ap_bytes(inst.outs[0])
                elif base.startswith("wq"):
                    dma_bytes["wq"] += ap_bytes(inst.outs[0])
                elif base.startswith("out") or ref.startswith("out"):
                    dma_bytes["out"] += ap_bytes(inst.ins[0])
            elif cls == "InstMatmult":
                base = _re.sub(r"_\d+$", "", inst.ins[0].memref)
                if base.startswith("warmsrc"):
                    mm_quarters.append(-2)
                elif base.startswith("wqf"):
                    mm_quarters.append(0)
                else:
                    mm_quarters.append(quarter_order.get(base, -1))

    exp = {
        "xts": KDIM * M * 2,
        "wqf": 128 * 1024 * KT * 2,
        "wq": (NQ * u - 1) * 128 * KT * 1024 * 2,
        "out": NQ * M * 1024 * 2,
    }
    for k, v in exp.items():
        if dma_bytes[k] != v:
            probs.append(f"dma bytes {k}: {dma_bytes[k]} != {v}")
    n_warm = sum(1 for q in mm_quarters if q == -2)
    if n_warm and max(i for i, q in enumerate(mm_quarters) if q == -2) >= n_warm:
        probs.append("warm-up matmuls not scheduled first")
    mm_quarters = [q for q in mm_quarters if q != -2]
    if len(mm_quarters) != 32 * r_total * NQ:
        probs.append(f"mm count {len(mm_quarters)}")
    if any(q < 0 for q in mm_quarters):
        probs.append("mm with unknown operand")
    last_idx = {}
    first_idx = {}
    for i, q in enumerate(mm_quarters):
        last_idx[q] = i
        first_idx.setdefault(q, i)
    nq_total = NQ * u
    for q in range(nq_total):
        for qp in range(q - 2):
            if qp in last_idx and q in first_idx and first_idx[q] < last_idx[qp]:
                probs.append(f"quarter {q} starts before quarter {qp} ends")
                return probs
    return probs


# ------------------------------------------------------------------ planner

def _core_time(u, r):
    """Predicted core time (us): max of PE and DMA cost (fp16 calibration)."""
    return max(28.4 * r + 5.0, 22.4 * u + 8.0 * r + 15.0)


def _anneal(plan, rng, iters=60000):
    """Refine a per-core [(expert, rows_tuple)] assignment by moving whole
    groups or row-slices between cores, minimizing a smooth max of the
    predicted per-core times."""

    def cost(g):
        return _core_time(len(g), sum(len(rr) for _, rr in g)) if g else 1000.0

    def full_score(p):
        costs = np.array([cost(g) for g in p])
        sigs = {tuple(sorted(len(rr) for _, rr in g)) for g in p}
        return 8.0 * np.log(np.exp(costs / 8.0).sum()) + 0.2 * len(sigs)

    plan = [[(e, tuple(rr)) for e, rr in g] for g in plan]
    cur = full_score(plan)
    best_plan, best = [list(g) for g in plan], max(cost(g) for g in plan)
    for it in range(iters):
        temp = max(0.02, 2.0 * (1 - it / iters))
        p = [list(g) for g in plan]
        a = int(rng.integers(0, len(p)))
        if not p[a]:
            continue
        gi = int(rng.integers(0, len(p[a])))
        e, rows = p[a][gi]
        bb = int(rng.integers(0, len(p)))
        if bb == a:
            continue
        if rng.random() < 0.5 or len(rows) < 2:
            p[a].pop(gi)
            p[bb].append((e, rows))
        else:
            k = int(rng.integers(1, len(rows)))
            p[a][gi] = (e, rows[:k])
            p[bb].append((e, rows[k:]))
        merged = {}
        for ee, rr in p[bb]:
            merged[ee] = merged.get(ee, ()) + rr
        p[bb] = [(ee, rr) for ee, rr in merged.items()]
        if sum(len(rr) for _, rr in p[bb]) > MAX_ROWS:
            continue
        sc = full_score(p)
        if sc < cur or rng.random() < np.exp((cur - sc) / (temp * 4.0)):
            plan, cur = p, sc
            tm = max(cost(g) for g in p)
            if tm < best:
                best, best_plan = tm, [list(g) for g in p]
    return [[(e, list(rr)) for e, rr in g] for g in best_plan]


def _plan_uniform(cat_ids, n_cores=NCORES):
    """Try to give every core the SAME (r1, r2, ...) signature so all
    cores run one byte-identical program: single compile, single schedule
    draw (quality-gated), homogeneous timing. Returns a per-core plan or
    None if no uniform signature fits this cat distribution."""
    experts = {}
    for i, c in enumerate(np.asarray(cat_ids).tolist()):
        experts.setdefault(int(c), []).append(i)
    counts = {e: len(r) for e, r in experts.items()}

    def partitions(n, maxp):
        if n == 0:
            yield ()
            return
        for p in range(min(n, maxp), 0, -1):
            for rest in partitions(n - p, p):
                yield (p,) + rest

    # preference: fewer slots (less W traffic), then larger first slot
    # (longer first-quarter startup coverage)
    cands = sorted(
        (sig for sig in partitions(8, 8) if 2 <= len(sig) <= 4),
        key=lambda s: (len(s), -s[0]),
    )

    def try_sig(sig):
        from collections import Counter
        avail = Counter()
        for s in sig:
            avail[s] += n_cores
        sizes = sorted(set(sig), reverse=True)
        order = sorted(counts, key=lambda e: -counts[e])

        def comps(c):
            out = []

            def rec(rem, maxs, cur):
                if rem == 0:
                    out.append(tuple(cur))
                    return
                for s in [x for x in sizes if x <= maxs and x <= rem]:
                    rec(rem - s, s, cur + [s])

            rec(c, sizes[0], [])
            return out

        assign = {}

        def bt(idx):
            if idx == len(order):
                return all(v == 0 for v in avail.values())
            e = order[idx]
            for comp in comps(counts[e]):
                cc = {}
                for s in comp:
                    cc[s] = cc.get(s, 0) + 1
                if all(avail[s] >= n for s, n in cc.items()):
                    for s, n in cc.items():
                        avail[s] -= n
                    assign[e] = comp
                    if bt(idx + 1):
                        return True
                    for s, n in cc.items():
                        avail[s] += n
                    del assign[e]
            return False

        if not bt(0):
            return None
        # cut each expert's rows into its chunks
        chunks = {s: [] for s in sizes}
        for e in order:
            pos = 0
            for s in assign[e]:
                chunks[s].append((e, experts[e][pos:pos + s]))
                pos += s
        # pair one chunk per sig slot per core, avoiding duplicate experts
        # within a core (which would double-load that expert's W)
        import itertools
        cores = [[] for _ in range(n_cores)]
        used = {s: [False] * len(chunks[s]) for s in chunks}
        ok = True
        for ci in range(n_cores):
            seen = set()
            for s in sig:
                pick = None
                for j, (e, rows) in enumerate(chunks[s]):
                    if not used[s][j] and e not in seen:
                        pick = j
                        break
                if pick is None:
                    ok = False
                    break
                used[s][pick] = True
                e, rows = chunks[s][pick]
                seen.add(e)
                cores[ci].append((e, rows))
            if not ok:
                break
        if not ok:
            return None
        return cores

    for sig in cands:
        plan = try_sig(sig)
        if plan is not None:
            return plan
    return None


def _plan_assignment(cat_ids, n_cores=NCORES, iters=2500, seed=0):
    """Greedy randomized assignment of (expert, row-chunk) groups to cores,
    minimizing the predicted max per-core time. Returns per-core list of
    (expert, row_indices)."""
    uni = _plan_uniform(cat_ids, n_cores)
    if uni is not None:
        return uni
    experts = {}
    for i, c in enumerate(np.asarray(cat_ids).tolist()):
        experts.setdefault(int(c), []).append(i)
    items = sorted(experts.items(), key=lambda kv: -len(kv[1]))
    rng = np.random.default_rng(seed)

    best, best_cost = None, float("inf")
    for attempt in range(iters):
        cores = [[] for _ in range(n_cores)]
        rows_c = [0] * n_cores
        u_c = [0] * n_cores
        ok = True
        if attempt == 0:
            order, cap = items, 8
        else:
            order = list(items)
            rng.shuffle(order)
            cap = int(rng.integers(5, 9))
        for e, rows in order:
            rem = list(rows)
            while rem:
                take = min(len(rem), cap)
                cand, cand_cost = None, float("inf")
                for c in range(n_cores):
                    for t in range(take, 0, -1):
                        if rows_c[c] + t <= 8:
                            cost = _core_time(u_c[c] + 1, rows_c[c] + t) - 0.01 * t
                            if cost < cand_cost:
                                cand_cost, cand = cost, (c, t)
                            break
                if cand is None:
                    ok = False
                    break
                c, t = cand
                cores[c].append((e, rem[:t]))
                rem = rem[t:]
                rows_c[c] += t
                u_c[c] += 1
            if not ok:
                break
        if not ok:
            continue
        cost = max(
            _core_time(len(g), sum(len(r) for _, r in g)) for g in cores if g
        )
        if any(not g for g in cores):
            cost += 1000.0
        sigs = {tuple(sorted(len(r) for _, r in g)) for g in cores}
        cost += 0.3 * len(sigs)
        if cost < best_cost:
            best_cost, best = cost, [list(g) for g in cores]

    assert best is not None, "planner failed to place rows"
    best = _anneal(best, rng)
    return [sorted(g, key=lambda er: -len(er[1])) for g in best]


# ------------------------------------------------------------------- runner

def _install_compile_cache():
    from concourse import bass2jax

    bass2jax.install_neuronx_cc_hook()
    import libneuronxla

    if getattr(libneuronxla, "_memo_wrapped", False):
        return
    inner = libneuronxla.neuronx_cc

    def memo_cc(code, code_format, platform_version, file_prefix):
        try:
            os.makedirs(_NEFF_CACHE_DIR, exist_ok=True)
            key = hashlib.sha256(
                code + b"|" + code_format + b"|" + str(platform_version).encode()
            ).hexdigest()
            path = os.path.join(_NEFF_CACHE_DIR, key + ".pkl")
            if os.path.exists(path):
                with open(path, "rb") as f:
                    return pickle.load(f)
        except Exception:
            path = None
        r = inner(code, code_format, platform_version, file_prefix)
        if path is not None:
            try:
                with open(path, "wb") as f:
                    pickle.dump(r, f)
            except Exception:
                pass
        return r

    libneuronxla.neuronx_cc = memo_cc
    libneuronxla._memo_wrapped = True


def _make_exec(nc):
    import jax
    from concourse.bass2jax import _bass_exec_p

    in_names, out_names, out_avals, zero_outs = [], [], [], []
    for alloc in nc.m.functions[0].allocations:
        if not isinstance(alloc, mybir.MemoryLocationSet):
            continue
        name = alloc.memorylocations[0].name
        if alloc.kind == "ExternalInput":
            in_names.append(name)
        elif alloc.kind == "ExternalOutput":
            out_names.append(name)
            shape = tuple(alloc.tensor_shape)
            dtype = mybir.dt.np(alloc.dtype)
            out_avals.append(jax.core.ShapedArray(shape, dtype))
            zero_outs.append(np.zeros(shape, dtype))
    n_params = len(in_names)
    all_names = tuple(in_names + out_names)

    def _body(*args):
        outs = _bass_exec_p.bind(
            *args,
            out_avals=tuple(out_avals),
            in_names=all_names,
            out_names=tuple(out_names),
            lowering_input_output_aliases=(),
            sim_require_finite=True,
            sim_require_nnan=True,
            nc=nc,
        )
        return tuple(outs)

    donate = tuple(range(n_params, n_params + len(out_names)))
    jit = jax.jit(_body, donate_argnums=donate, keep_unused=True)
    return jit, in_names, out_names, zero_outs


def _run_many(execs, in_maps):
    import jax

    devices = jax.devices()[: len(execs)]
    launches = []
    for c, (jit, in_names, out_names, zero_outs) in enumerate(execs):
        args = [
            jax.device_put(np.ascontiguousarray(in_maps[c][n]), devices[c])
            for n in in_names
        ]
        zs = [jax.device_put(z, devices[c]) for z in zero_outs]
        launches.append((jit, args, zs, out_names))
    outs = [jit(*args, *zs) for jit, args, zs, _ in launches]
    return [
        {name: np.asarray(a) for name, a in zip(out_names, o)}
        for (_, _, _, out_names), o in zip(launches, outs)
    ]


# ------------------------------------------------------------------- kernel

_EXEC_CACHE = {}
_PLAN_CACHE = {}


def _get_exec(group_rows, dtype=FP16):
    key = (tuple(group_rows), str(dtype))
    if key not in _EXEC_CACHE:
        nc = None
        for salt in range(6):
            nc = _build_program(group_rows, dtype, salt=salt)
            probs = _validate_nc(nc, group_rows)
            if not probs:
                break
        _EXEC_CACHE[key] = _make_exec(nc)
    return _EXEC_CACHE[key]


def kernel(x, cat_ids, W, b):
    _install_compile_cache()

    x = np.asarray(x, dtype=np.float32)
    cat_np = np.asarray(cat_ids).astype(np.int64)
    W = np.asarray(W, dtype=np.float32)
    b = np.asarray(b, dtype=np.float32)
    B = x.shape[0]
    assert x.shape == (B, SEQ, KDIM) and W.shape == (16, KDIM, NDIM)

    pkey = cat_np.tobytes()
    if pkey not in _PLAN_CACHE:
        _PLAN_CACHE[pkey] = _plan_assignment(cat_np)
    plan = _PLAN_CACHE[pkey]

    np_dt = mybir.dt.np(FP16)
    # Pre-layout each needed expert's W as [4 quarters, 128 p, 8 kk * 1024 f]
    # fp16 (k = kk*128 + p), so the kernel-side W DMAs are plain 2D copies
    # with 16KB-contiguous rows.
    needed = sorted({cn for groups in plan for cn, _ in groups})
    wprep = {}
    for cn in needed:
        wprep[cn] = np.ascontiguousarray(
            W[cn].reshape(KT, 128, NQ, 1024).transpose(2, 1, 0, 3)
        ).astype(np_dt).reshape(NQ * 128, KT * 1024)

    execs, in_maps, row_lists = [], [], []
    for groups in plan:
        sig = tuple(len(rr) for _, rr in groups)
        execs.append(_get_exec(sig, FP16))
        rows = [i for _, rr in groups for i in rr]
        xt = np.ascontiguousarray(
            x[rows].transpose(2, 0, 1).reshape(KDIM, SEQ * len(rows))
        ).astype(np_dt)
        w = np.concatenate([wprep[cn] for cn, _ in groups], axis=0)
        in_maps.append({"xt": xt, "w": w})
        row_lists.append(rows)

    results = _run_many(execs, in_maps)

    out = np.empty((B, SEQ, NDIM), dtype=np.float32)
    for rows, res in zip(row_lists, results):
        r = len(rows)
        o = res["out"].reshape(NQ, r, SEQ, 1024)
        out[rows] = np.moveaxis(o, 0, 2).reshape(r, SEQ, NDIM)
    out += b[cat_np][:, None, :]
    return out
